# revision 78
# baseline (speedup 1.0000x reference)
"""GraphSAGE 2-layer forward on 8 TRN2 NeuronCores — v14.

Design (dst nodes sharded 6250/core, padded to D=6272 columns):
- L1 aggregation: messages x[src] are HOST-pre-gathered (fp8) into a
  per-core slot-ordered stream table [128, S1/128, 128], streamed with
  line-rate HWDGE DMAs — zero SWDGE descgen. Segment-sum via flipped
  one-hot matmuls: stationary = 128-slot msg group [slot, feat], moving
  = narrow {0,1} segment matrix oh[slot, dstcol] -> PSUM window
  [feat, <=512 dstcols]; 1/deg applied after aggregation (DVE mult).
- p = h1 @ W2_l (64-col fp8 rows) computed per closing window; p row
  slices AllGathered incrementally (3 slices fired as their windows
  close) so remote-pass descriptor generation starts mid-L1.
- L2 aggregation: dma_gather of 256B elements (= PAIR of packed fp8
  p rows) over 4 SWDGE queues (Q7 descgen is the kernel bottleneck,
  ~3 ns/edge); slots sorted (window, src-parity, dst) so each
  microchunk's stationary is the matching 64-col half. 6 passes:
  self/remote x p-slice; last passes sweep windows in REVERSE so final
  window closes spread out. Dense o2 = h1@W2_r + b2 precomputed;
  per-window PE transpose of mean2T, Exp/Ln + bias-subtract batches,
  incremental output DMA.
"""

import numpy as np
import ml_dtypes

import concourse.bacc as bacc
import concourse.bass as bass
import concourse.mybir as mybir
import concourse.tile as tile
from concourse.bass_utils import run_bass_kernel_spmd

N = 50000
F = 128
HID = 256
CLS = 47
CORES = 8
NPC = N // CORES            # 6250
D = 6272                    # padded columns per core (49*128)
CH = 4096                   # slots per L1 stream chunk
CH2 = 2048                  # slots per L2 gather chunk (best descgen rate)
MC = 128                    # slots per matmul microchunk
WIN = 512                   # psum window columns
SPLIT = 32768               # int16 gather index limit
D1 = 3072                   # p split: windows 0-5
D2 = D - D1                 # 3200: windows 6-12
# p row-slices for the incremental AllGather (aligned to 512-col windows)
NQ = 3
QS = [512, 2560, 3200]
QOFF = [0, 512, 3072]
QWIN = [(0,), (1, 2, 3, 4, 5), (6, 7, 8, 9, 10, 11, 12)]

f32 = mybir.dt.float32
bf16 = mybir.dt.bfloat16
f8 = mybir.dt.float8e4
i16 = mybir.dt.int16
F8 = ml_dtypes.float8_e4m3
ALU = mybir.AluOpType
ACTF = mybir.ActivationFunctionType

IDENT_F32 = np.eye(128, dtype=np.float32)
NW = (D + WIN - 1) // WIN   # windows (last one narrower)


def _win_width(w):
    return min(WIN, D - w * WIN)


def _layer_struct(idx_slots, col_slots, val_slots, S, S_half, build_idx=True,
                  ch=CH, oh_dtype=ml_dtypes.bfloat16, par_slots=None,
                  opener_passes=None):
    """Uniform chunk/microchunk/emission structure + per-core idx/oh.
    S_half: list of per-pass padded slot counts (any number of passes).
    ch: chunk size, int or per-pass list.
    par_slots: optional per-slot parity [CORES, S] — slot's data sits in
    the 64-col half `parity` of its 128-col gathered element (packed-pair
    tables). Emissions then carry pk and the matmul uses that stationary
    half. Without it pk=None and the full 128-col stationary is used."""
    if isinstance(ch, int):
        ch = [ch] * len(S_half)
    offs = np.concatenate([[0], np.cumsum(S_half)])
    chunk_list = []
    for h in range(len(S_half)):
        off = int(offs[h])
        s0 = 0
        while s0 < S_half[h]:
            n = min(ch[h], S_half[h] - s0)
            chunk_list.append((h, off + s0, n))
            s0 += n
    n_mc = S // MC
    mc_half = np.zeros(n_mc, np.int64)
    for h in range(len(S_half)):
        off = int(offs[h])
        mc_half[off // MC:(off + S_half[h]) // MC] = h

    # per-(mc, parity-class) col ranges, unioned over cores
    pks = (0, 1) if par_slots is not None else (None,)
    cmin = {p: np.full(n_mc, 1 << 30, np.int64) for p in pks}
    cmax = {p: np.full(n_mc, -1, np.int64) for p in pks}
    for c in range(CORES):
        cs = col_slots[c].reshape(n_mc, MC)
        for p in pks:
            valid = cs >= 0
            if p is not None:
                valid &= par_slots[c].reshape(n_mc, MC) == p
            anyv = valid.any(axis=1)
            lo = np.where(valid, cs, 1 << 30).min(axis=1)
            hi = np.where(valid, cs, -1).max(axis=1)
            cmin[p] = np.minimum(cmin[p], np.where(anyv, lo, cmin[p]))
            cmax[p] = np.maximum(cmax[p], np.where(anyv, hi, cmax[p]))

    emissions = []
    oh_off = 0
    win_first, win_last = {}, {}
    eid = 0
    for m in range(n_mc):
        ems = []
        h = int(mc_half[m])
        for p in pks:
            if cmax[p][m] < 0:
                continue
            w0, w1 = int(cmin[p][m]) // WIN, int(cmax[p][m]) // WIN
            for w in range(w0, w1 + 1):
                ww = _win_width(w)
                key = (h, w)
                full = False
                if key not in win_first:
                    win_first[key] = eid
                    full = opener_passes is not None and h in opener_passes
                if full:
                    c0, cw = w * WIN, ww
                else:
                    c0 = max(int(cmin[p][m]), w * WIN)
                    cw = min(int(cmax[p][m]), w * WIN + ww - 1) - c0 + 1
                win_last[key] = eid
                ems.append((w, c0, cw, oh_off, p))
                oh_off += cw
                eid += 1
        emissions.append(ems)
    OHW = oh_off
    op_set = set(win_first.values())
    cl_set = {v: k for k, v in win_last.items()}
    # role per (pass, window) closer: copy / add / final
    by_win = {}
    for (h, w) in win_first:
        by_win.setdefault(w, []).append(h)
    role = {}
    for w, hs in by_win.items():
        hs.sort()
        for i, h in enumerate(hs):
            role[(h, w)] = ("copy" if i == 0 else
                            ("final" if i == len(hs) - 1 else "add"))
            if len(hs) == 1:
                role[(h, w)] = "copy_final"
    sched_mcs = []
    eid = 0
    for m in range(n_mc):
        lst = []
        for (w, c0, cw, off, p) in emissions[m]:
            cl = cl_set.get(eid)
            lst.append(dict(win=w, c0=c0, cw=cw, off=off, pk=p,
                            opener=(eid in op_set), closer=cl,
                            role=role.get(cl) if cl else None))
            eid += 1
        sched_mcs.append(lst)

    idx_all, oh_all = [], []
    for c in range(CORES):
        oh = np.zeros((128, OHW), np.float32)
        cs = col_slots[c].reshape(n_mc, MC)
        vs = val_slots[c].reshape(n_mc, MC)
        for m in range(n_mc):
            for e in sched_mcs[m]:
                rel = cs[m] - e["c0"]
                ok = (cs[m] >= 0) & (rel >= 0) & (rel < e["cw"])
                if e["pk"] is not None:
                    ok &= par_slots[c].reshape(n_mc, MC)[m] == e["pk"]
                p_idx = np.arange(MC)[ok]
                oh[p_idx, e["off"] + rel[ok]] = vs[m][ok]
        oh_all.append(np.ascontiguousarray(oh.astype(oh_dtype)))
        if build_idx:
            a = idx_slots[c]
            wrp = np.zeros((128, S // 16), np.int16)
            b = a.reshape(S // 16, 16).T.astype(np.int16)
            for g in range(8):
                wrp[16 * g:16 * (g + 1), :] = b
            idx_all.append(wrp)

    return dict(S=S, S_half=S_half, OHW=OHW, chunks=chunk_list,
                mcs=sched_mcs, n_mc=n_mc, mc_half=mc_half), idx_all, oh_all


def host_prep_all(x, edge_index):
    src = np.asarray(edge_index[0], np.int64)
    dst = np.asarray(edge_index[1], np.int64)
    deg = np.bincount(dst, minlength=N).astype(np.int64)
    invdeg = (1.0 / np.maximum(deg, 1)).astype(np.float32)
    core = dst // NPC

    def build_slots_multi(tab_idx, passid, npass, parity=None,
                          rev_passes=()):
        per = {}
        S_half = [0] * npass
        for c in range(CORES):
            for h in range(npass):
                m = (core == c) & (passid == h)
                ti = tab_idx[m]
                dl = dst[m] - c * NPC
                wkey = -(dl // WIN) if h in rev_passes else dl // WIN
                if parity is None:
                    o = np.argsort(dl, kind="stable")
                    pr = np.zeros(len(ti), np.int64)
                else:
                    pr = parity[m]
                    # cluster parity within each dst window so microchunks
                    # are (mostly) parity-pure
                    o = np.lexsort((dl, pr, wkey))
                per[(c, h)] = (ti[o], dl[o], invdeg[dl[o] + c * NPC],
                               pr[o])
        for h in range(npass):
            mx = max(len(per[(c, h)][0]) for c in range(CORES))
            S_half[h] = ((mx + MC - 1) // MC) * MC
        S = int(sum(S_half))
        offs = np.concatenate([[0], np.cumsum(S_half)]).astype(np.int64)
        idx_slots = np.zeros((CORES, S), np.int64)
        col_slots = np.full((CORES, S), -1, np.int64)
        val_slots = np.zeros((CORES, S), np.float32)
        par_slots = np.ones((CORES, S), np.int64)
        for c in range(CORES):
            for h in range(npass):
                ti, dl, iv, pr = per[(c, h)]
                off = int(offs[h])
                n = len(ti)
                idx_slots[c, off:off + n] = ti
                col_slots[c, off:off + n] = dl
                val_slots[c, off:off + n] = iv
                par_slots[c, off:off + n] = pr
        return idx_slots, col_slots, val_slots, S, S_half, par_slots

    # L1: single pass (messages are host-pre-gathered, no idx-width limit).
    # oh1 holds exact {0,1} (fp8-safe); 1/deg is applied after aggregation.
    i1, c1, v1, S1, S1h, _ = build_slots_multi(src, np.zeros_like(src), 1)
    v1 = (c1 >= 0).astype(np.float32)
    l1, _, oh1_all = _layer_struct(i1, c1, v1, S1, S1h, build_idx=False,
                                   oh_dtype=F8, opener_passes={0})
    l1["invdeg"] = invdeg

    # L2: 6 passes (self slice a-c, remote slice a-c); slices match
    # the sliced p AllGathers so remote descgen starts as slices land.
    # p tables are packed [*, 64]; a 256B gather element covers a row PAIR,
    # so the gather idx is tabidx//2 and tabidx%2 picks the stationary half.
    owner = src // NPC
    row = src % NPC
    qid = np.digitize(row, QOFF[1:] + [NPC])
    qrow = row - np.asarray(QOFF)[qid]
    qsz = np.asarray(QS)[qid]
    selfmask = (owner == core)
    # pass ids: self-a=0, self-b=1, rem-a=2, rem-b=3, rem-c=4, self-c=5.
    # self-c LAST so final window closes spread along rem-c's sweep.
    self_pid = np.asarray([0, 1, 5])[qid]
    passid = np.where(selfmask, self_pid, 2 + qid)
    tabidx = np.where(selfmask, qrow, owner * qsz + qrow)
    i2, c2, v2, S2, S2h, p2 = build_slots_multi(
        tabidx // 2, passid, 2 * NQ, parity=tabidx % 2,
        rev_passes={3, 4, 5})
    v2 = (c2 >= 0).astype(np.float32)
    l2, idx2_all, oh2_all = _layer_struct(
        i2, c2, v2, S2, S2h,
        ch=[256, 512, 512, CH2, CH2, 512],
        par_slots=p2, oh_dtype=F8, opener_passes={2, 3, 4})
    return l1, l2, i1, oh1_all, idx2_all, oh2_all


def _max_chunk_ohw(lx):
    best = 0
    for (h, s0, nsl) in lx["chunks"]:
        ems = [e for m in range(s0 // MC, (s0 + nsl) // MC)
               for e in lx["mcs"][m]]
        if ems:
            best = max(best, ems[-1]["off"] + ems[-1]["cw"] - ems[0]["off"])
    return best


def _build(l1, l2):
    OHT1 = ((_max_chunk_ohw(l1) + 255) // 256) * 256
    OHT2 = ((_max_chunk_ohw(l2) + 255) // 256) * 256
    nc = bacc.Bacc("TRN2", num_devices=CORES, num_swdge_queues=4)
    msg1_h = nc.declare_dram_parameter("msg1", [128, l1["S"] // 128, F],
                                       f8, False)
    idx2_h = nc.declare_dram_parameter("idx2", [128, l2["S"] // 16], i16, False)
    oh1_h = nc.declare_dram_parameter("oh1", [128, l1["OHW"]], f8, False)
    oh2_h = nc.declare_dram_parameter("oh2", [128, l2["OHW"]], f8, False)
    ivd_h = nc.declare_dram_parameter("ivd", [128, D], bf16, False)
    xtc_h = nc.declare_dram_parameter("xtc", [128, D], bf16, False)
    w1l_h = nc.declare_dram_parameter("w1l", [F, HID], bf16, False)
    w1r_h = nc.declare_dram_parameter("w1r", [F, HID], bf16, False)
    w2l_h = nc.declare_dram_parameter("w2l", [128, 2 * CLS], bf16, False)
    w2r_h = nc.declare_dram_parameter("w2r", [128, 2 * CLS], bf16, False)
    b1_h = nc.declare_dram_parameter("b1c", [128, 2], f32, False)
    b2_h = nc.declare_dram_parameter("b2r", [1, CLS], f32, False)
    ident_h = nc.declare_dram_parameter("ident", [128, 128], f32, False)
    out_h = nc.declare_dram_parameter("out", [D, CLS], bf16, True)

    p_loc = [nc.dram_tensor(f"p_loc_{q}", [QS[q], 128], f8)
             for q in range(NQ)]
    p_full = [nc.dram_tensor(f"p_full_{q}", [CORES * QS[q], 128], f8)
              for q in range(NQ)]

    j_chunks = [(j * WIN, _win_width(j)) for j in range(NW)]

    with tile.TileContext(nc) as tc:
        with (
            tc.tile_pool(name="const", bufs=1) as cp,
            tc.tile_pool(name="msg", bufs=6) as msgp,
            tc.tile_pool(name="oh", bufs=3) as ohp,
            tc.tile_pool(name="msg2", bufs=8) as msgp2,
            tc.tile_pool(name="msgc", bufs=12) as msgp3,
            tc.tile_pool(name="oh2", bufs=5) as ohp2,
            tc.tile_pool(name="sm", bufs=4) as smp,
        ):
            idx2_sb = cp.tile([128, l2["S"] // 16], i16, tag="idx2")
            w1l_sb = cp.tile([F, HID], bf16, tag="w1l")
            nc.sync.dma_start(w1l_sb[:], w1l_h[:, :])
            w1r_sb = cp.tile([F, HID], bf16, tag="w1r")
            nc.sync.dma_start(w1r_sb[:], w1r_h[:, :])
            w2l_sb = cp.tile([128, 2 * CLS], bf16, tag="w2l")
            nc.sync.dma_start(w2l_sb[:], w2l_h[:, :])
            w2r_sb = cp.tile([128, 2 * CLS], bf16, tag="w2r")
            nc.sync.dma_start(w2r_sb[:], w2r_h[:, :])
            b1_sb = cp.tile([128, 2], f32, tag="b1")
            nc.sync.dma_start(b1_sb[:], b1_h[:, :])
            b2_sb = cp.tile([1, CLS], f32, tag="b2")
            nc.sync.dma_start(b2_sb[:], b2_h[:, :])
            ident = cp.tile([128, 128], f32, tag="ident")
            nc.sync.dma_start(ident[:], ident_h[:, :])
            ones_sb = cp.tile([1, 128], f32, tag="ones")
            nc.vector.memset(ones_sb[:], 1.0)
            identb = cp.tile([128, 128], bf16, tag="identb")
            nc.vector.tensor_copy(identb[:], ident[:])

            h1T = cp.tile([128, 2, D], bf16, tag="h1T")

            def agg_layer(lx, idx_sb, oh_h, tables, meanT, mean_lo, winp,
                          on_hi_close=None, stream=None, dt=bf16, scale=None,
                          pools=None, ohtw=None, on_pass_end=None,
                          npo=128, selem=128, defer_pass=None,
                          defer_pool=None):
                chunks, mcs = lx["chunks"], lx["mcs"]
                mp, op_ = pools
                chmax = max(n for (_, _, n) in chunks)
                win_tiles = {}

                def do_emissions(msg, h, s0, nsl):
                    ems = [e for m in range(s0 // MC, (s0 + nsl) // MC)
                           for e in mcs[m]]
                    if not ems:
                        return
                    o0 = ems[0]["off"]
                    o1 = ems[-1]["off"] + ems[-1]["cw"]
                    ohw = o1 - o0
                    assert ohw <= ohtw, ohw
                    oht = op_.tile([128, ohtw], dt, tag="o")
                    nc.scalar.dma_start(oht[:, 0:ohw], oh_h[:, o0:o1])
                    for mi, m in enumerate(range(s0 // MC, (s0 + nsl) // MC)):
                        for e in mcs[m]:
                            w = e["win"]
                            key = (h, w)
                            first = False
                            if e["opener"]:
                                win_tiles[key] = winp.tile(
                                    [128, WIN], f32, tag="win",
                                    name=f"win_{h}_{w}")
                                if e["cw"] == _win_width(w):
                                    first = True   # full-width PE opener
                                else:
                                    nc.vector.memset(
                                        win_tiles[key][0:npo,
                                                       0:_win_width(w)], 0.0)
                            ps = win_tiles[key]
                            rel = e["c0"] - w * WIN
                            pk = e["pk"]
                            stat = (msg[:, mi, :] if pk is None
                                    else msg[:, mi, 128 * pk:128 * pk + 64])
                            nc.tensor.matmul(
                                ps[0:npo, rel:rel + e["cw"]],
                                stat,
                                oht[:, e["off"] - o0:e["off"] - o0 + e["cw"]],
                                start=first,
                                stop=(e["closer"] is not None))
                            if e["closer"] is not None:
                                hh, ww = e["closer"]
                                wid = _win_width(ww)
                                cws = slice(ww * WIN, ww * WIN + wid)
                                role = e["role"]
                                if role == "copy":
                                    nc.scalar.activation(
                                        mean_lo[0:npo, cws], ps[0:npo, 0:wid],
                                        ACTF.Copy)
                                elif role == "add":
                                    nc.vector.tensor_tensor(
                                        mean_lo[0:npo, cws], ps[0:npo, 0:wid],
                                        mean_lo[0:npo, cws], ALU.add)
                                elif role == "copy_final":
                                    if scale is not None:
                                        nc.vector.tensor_tensor(
                                            meanT[0:npo, cws],
                                            ps[0:npo, 0:wid],
                                            scale[0:npo, cws], ALU.mult)
                                    else:
                                        nc.scalar.activation(
                                            meanT[0:npo, cws],
                                            ps[0:npo, 0:wid], ACTF.Copy)
                                    if on_hi_close is not None:
                                        on_hi_close(ww)
                                else:  # final
                                    nc.vector.tensor_tensor(
                                        meanT[0:npo, cws], ps[0:npo, 0:wid],
                                        mean_lo[0:npo, cws], ALU.add)
                                    if scale is not None:
                                        nc.vector.tensor_tensor(
                                            meanT[0:npo, cws],
                                            meanT[0:npo, cws],
                                            scale[0:npo, cws], ALU.mult)
                                    if on_hi_close is not None:
                                        on_hi_close(ww)
                                del win_tiles[key]

                deferred = []
                order = list(range(len(chunks)))
                if defer_pass is not None:
                    # issue gathers: self (0,1), deferred self-c (gated on
                    # p_loc_c ~ same time as the AG triggers), then remote
                    order = ([ci for ci, c in enumerate(chunks)
                              if c[0] in (0, 1)] +
                             [ci for ci, c in enumerate(chunks)
                              if c[0] == defer_pass] +
                             [ci for ci, c in enumerate(chunks)
                              if c[0] in (2, 3, 4)])
                for ci in order:
                    (h, s0, nsl) = chunks[ci]
                    if h == defer_pass:
                        msg = defer_pool.tile([128, nsl // 128, selem], dt,
                                              tag="m", name=f"dm_{ci}")
                    else:
                        msg = mp.tile([128, chmax // 128, selem], dt, tag="m")
                    if stream is not None:
                        nc.sync.dma_start(
                            msg[:, 0:nsl // 128, :],
                            stream[:, s0 // 128:(s0 + nsl) // 128, :])
                    else:
                        nc.gpsimd.dma_gather(
                            msg[:, 0:nsl // 128, :], tables[h],
                            idx_sb[:, s0 // 16:(s0 + nsl) // 16],
                            nsl, nsl, selem, single_packet=False,
                            queue_num=ci % 4)
                    if on_pass_end is not None and (
                            ci + 1 == len(chunks) or chunks[ci + 1][0] != h):
                        on_pass_end(h)
                    if h == defer_pass:
                        deferred.append((msg, h, s0, nsl))
                        continue
                    do_emissions(msg, h, s0, nsl)
                for (msg, h, s0, nsl) in deferred:
                    do_emissions(msg, h, s0, nsl)

            # =============== Layer 1 ===============
            with (
                tc.tile_pool(name="l1", bufs=1) as l1p,
                tc.tile_pool(name="pp", bufs=2, space="PSUM") as ppp,
            ):
                meanT = l1p.tile([128, D], bf16, tag="meanT")
                xtc_sb = l1p.tile([128, D], bf16, tag="xtc")
                nc.scalar.dma_start(xtc_sb[:], xtc_h[:, :])
                ivd_sb = cp.tile([128, D], bf16, tag="ivd")
                nc.scalar.dma_start(ivd_sb[:], ivd_h[:, :])



                def l1_close(w):
                    j0, jn = w * WIN, _win_width(w)
                    for hh in (0, 1):
                        z = ppp.tile([128, WIN], f32, tag="z",
                                     name=f"z_{w}_{hh}")
                        nc.tensor.matmul(
                            z[:, 0:jn],
                            w1l_sb[:, hh * 128:(hh + 1) * 128],
                            meanT[:, j0:j0 + jn], start=True, stop=False)
                        nc.tensor.matmul(
                            z[:, 0:jn],
                            w1r_sb[:, hh * 128:(hh + 1) * 128],
                            xtc_sb[:, j0:j0 + jn], start=False, stop=True)
                        nc.scalar.activation(
                            h1T[:, hh, j0:j0 + jn], z[:, 0:jn], ACTF.Relu,
                            bias=b1_sb[:, hh:hh + 1], scale=1.0)
                    nt = jn // 128
                    pwin = smp.tile([128, 4, 128], f8, tag="psb",
                                    name=f"psb_{w}")
                    nc.vector.memset(pwin[:, 0:nt, :], 0.0)
                    for ti, t in enumerate(range(j0 // 128, (j0 + jn) // 128)):
                        ts = slice(t * 128, (t + 1) * 128)
                        pp_ps = ppp.tile([128, 64], f32, tag="pp",
                                         name=f"pp_{t}")
                        nc.tensor.matmul(pp_ps[:, 0:CLS], h1T[:, 0, ts],
                                         w2l_sb[:, 0:CLS], start=True,
                                         stop=False)
                        nc.tensor.matmul(pp_ps[:, 0:CLS], h1T[:, 1, ts],
                                         w2l_sb[:, CLS:2 * CLS], start=False,
                                         stop=True)
                        nc.scalar.activation(pwin[:, ti, 0:CLS],
                                             pp_ps[:, 0:CLS], ACTF.Copy)
                    wq = next(q for q in range(NQ) if w in QWIN[q])
                    dst = p_loc[wq].ap()[j0 - QOFF[wq]:j0 - QOFF[wq] + jn, :]
                    nc.sync.dma_start(
                        dst.rearrange("(t p) c -> p t c", p=128),
                        pwin[:, 0:nt, :])

                with tc.tile_pool(name="win", bufs=3, space="PSUM") as winp:
                    agg_layer(l1, None, oh1_h, None,
                              meanT, meanT, winp, on_hi_close=l1_close,
                              stream=msg1_h, dt=f8, scale=ivd_sb,
                              pools=(msgp, ohp), ohtw=OHT1)

                nc.scalar.dma_start(idx2_sb[:], idx2_h[:, :])
                b2_ps = ppp.tile([128, 64], f32, tag="pp")
                nc.tensor.matmul(b2_ps[:, 0:CLS], ones_sb[0:1, :],
                                 b2_sb[0:1, :], start=True, stop=True)
                b2bc = cp.tile([128, CLS], f32, tag="b2bc")
                nc.scalar.activation(b2bc[:], b2_ps[:, 0:CLS], ACTF.Copy)

                # L2 dense term h1 @ W2_r precomputed (overlaps AllGather)
                o2_all = cp.tile([128, (D // 128) * CLS], bf16, tag="o2a")
                for t in range(D // 128):
                    ts = slice(t * 128, (t + 1) * 128)
                    o_ps = ppp.tile([128, 64], f32, tag="pp",
                                    name=f"ops_{t}")
                    nc.tensor.matmul(o_ps[:, 0:CLS], h1T[:, 0, ts],
                                     w2r_sb[:, 0:CLS], start=True, stop=False)
                    nc.tensor.matmul(o_ps[:, 0:CLS], h1T[:, 1, ts],
                                     w2r_sb[:, CLS:2 * CLS], start=False,
                                     stop=True)
                    nc.scalar.activation(o2_all[:, t * CLS:(t + 1) * CLS],
                                         o_ps[:, 0:CLS], ACTF.Copy)

            # =============== Layer 2 ===============
            with (
                tc.tile_pool(name="l2", bufs=1) as l2p,
                tc.tile_pool(name="tp", bufs=2, space="PSUM") as tpp,
            ):
                mean2_lo = l2p.tile([128, D], bf16, tag="mean2_lo")
                mean2 = l2p.tile([128, D], bf16, tag="mean2")
                DT = D // 128
                lg2_all = l2p.tile([128, DT * CLS], bf16, tag="lg2a")
                ex_all = l2p.tile([128, DT * CLS], bf16, tag="exa")
                sm_all = l2p.tile([128, DT], f32, tag="sma")

                def ls_batch(wlo, whi):
                    t0, t1 = wlo * 4, min(whi * 4 + 4, DT)
                    ls_b = smp.tile([128, 25], f32, tag="ls",
                                    name=f"ls_{wlo}")
                    nc.scalar.activation(ls_b[:, 0:t1 - t0],
                                         sm_all[:, t0:t1], ACTF.Ln)
                    nls_b = smp.tile([128, 25], f32, tag="nls",
                                     name=f"nls_{wlo}")
                    nc.vector.tensor_scalar(nls_b[:, 0:t1 - t0],
                                            ls_b[:, 0:t1 - t0],
                                            -1.0, None, ALU.mult)
                    for w in range(wlo, whi + 1):
                        j0, jn = w * WIN, _win_width(w)
                        nt = jn // 128
                        res_w = smp.tile([128, 4, CLS], bf16, tag="res",
                                         name=f"res_{w}")
                        for ti, t in enumerate(range(w * 4, w * 4 + nt)):
                            cs = slice(t * CLS, (t + 1) * CLS)
                            nc.scalar.activation(
                                res_w[:, ti, :], lg2_all[:, cs],
                                ACTF.Identity,
                                bias=nls_b[:, t - t0:t - t0 + 1], scale=1.0)
                        nc.sync.dma_start(
                            out_h.ap()[j0:j0 + jn, :]
                            .rearrange("(t p) c -> p t c", p=128),
                            res_w[:, 0:nt, :])

                def l2_close(w):
                    j0, jn = w * WIN, _win_width(w)
                    nt = jn // 128
                    t0 = j0 // 128
                    for ti, t in enumerate(range(t0, t0 + nt)):
                        ts = slice(t * 128, (t + 1) * 128)
                        cs = slice(t * CLS, (t + 1) * CLS)
                        m2t = tpp.tile([128, 64], bf16, tag="tp",
                                       name=f"m2t_{t}")
                        nc.tensor.transpose(m2t[:, 0:CLS],
                                            mean2[0:CLS, ts],
                                            identb[0:CLS, 0:CLS])
                        m2s = smp.tile([128, CLS], f32, tag="m2s",
                                       name=f"m2s_{t}")
                        nc.vector.tensor_copy(m2s[:], m2t[:, 0:CLS])
                        nc.vector.tensor_tensor(lg2_all[:, cs],
                                                o2_all[:, cs],
                                                m2s[:], ALU.add)
                        nc.scalar.activation(ex_all[:, cs], lg2_all[:, cs],
                                             ACTF.Exp)
                        nc.vector.tensor_reduce(sm_all[:, t:t + 1],
                                                ex_all[:, cs],
                                                mybir.AxisListType.X, ALU.add)
                    if w == 6:
                        ls_batch(6, 12)
                    elif w == 0:
                        ls_batch(0, 5)

                cc_done = [False] * NQ

                def _ag(q):
                    if not cc_done[q]:
                        nc.gpsimd.collective_compute(
                            "AllGather", ALU.bypass,
                            replica_groups=[list(range(CORES))],
                            ins=[p_loc[q].ap().opt()],
                            outs=[p_full[q].ap().opt()])
                        cc_done[q] = True

                def fire_ag(h):
                    if h == 0:
                        _ag(0)
                    elif h == 1:
                        _ag(1)
                        _ag(2)

                with tc.tile_pool(name="win2", bufs=6, space="PSUM") as winp2:
                    ptabs = [p_loc[0], p_loc[1], p_full[0], p_full[1],
                             p_full[2], p_loc[2]]
                    pair = [t.ap().rearrange("(a b) c -> a (b c)", b=2)
                            for t in ptabs]
                    agg_layer(l2, idx2_sb, oh2_h,
                              tuple(pair),
                              mean2, mean2_lo, winp2, on_hi_close=l2_close,
                              dt=f8, scale=ivd_sb,
                              pools=(msgp2, ohp2), ohtw=OHT2,
                              on_pass_end=fire_ag, npo=64, selem=256,
                              defer_pass=5, defer_pool=msgp3)

    nc.compile()
    return nc


def _make_in_maps(inputs, l1, l2, i1_slots, oh1_all, idx2_all, oh2_all):
    x = np.asarray(inputs["x"], np.float32)
    xf8 = np.ascontiguousarray(x.astype(F8))
    invdeg = l1["invdeg"]
    w1l = np.ascontiguousarray(np.asarray(inputs["W1_l"], np.float32)
                               .astype(ml_dtypes.bfloat16))
    w1r = np.ascontiguousarray(np.asarray(inputs["W1_r"], np.float32)
                               .astype(ml_dtypes.bfloat16))
    w2lf = np.asarray(inputs["W2_l"], np.float32)
    w2rf = np.asarray(inputs["W2_r"], np.float32)
    w2l = np.ascontiguousarray(
        np.concatenate([w2lf[:128, :], w2lf[128:, :]], axis=1)
        .astype(ml_dtypes.bfloat16))
    w2r = np.ascontiguousarray(
        np.concatenate([w2rf[:128, :], w2rf[128:, :]], axis=1)
        .astype(ml_dtypes.bfloat16))
    b1c = np.ascontiguousarray(
        np.asarray(inputs["b1"], np.float32).reshape(2, 128).T)
    b2r = np.ascontiguousarray(
        np.asarray(inputs["b2"], np.float32).reshape(1, CLS))

    in_maps = []
    for c in range(CORES):
        xt = np.zeros((128, D), np.float32)
        xt[:, 0:NPC] = x[c * NPC:(c + 1) * NPC].T
        # host pre-gather: msg1[p, g, :] = x[src of slot g*128+p] (fp8)
        rows = xf8[i1_slots[c]]                        # [S1, F] fp8
        msg1 = np.ascontiguousarray(
            rows.reshape(l1["S"] // 128, 128, F).transpose(1, 0, 2))
        ivc = np.ones((D,), np.float32)
        ivc[0:NPC] = invdeg[c * NPC:(c + 1) * NPC]
        ivd = np.ascontiguousarray(
            np.broadcast_to(ivc.astype(ml_dtypes.bfloat16), (128, D)))
        in_maps.append({
            "msg1": msg1,
            "idx2": idx2_all[c],
            "oh1": oh1_all[c], "oh2": oh2_all[c],
            "ivd": ivd,
            "xtc": np.ascontiguousarray(xt.astype(ml_dtypes.bfloat16)),
            "w1l": w1l, "w1r": w1r, "w2l": w2l, "w2r": w2r,
            "b1c": b1c, "b2r": b2r, "ident": IDENT_F32,
        })
    return in_maps


def _run(inputs, trace=False, tmpdir=None):
    x = np.asarray(inputs["x"], np.float32)
    edge_index = np.asarray(inputs["edge_index"])
    l1, l2, idx1_all, oh1_all, idx2_all, oh2_all = host_prep_all(x, edge_index)
    nc = _build(l1, l2)
    in_maps = _make_in_maps(inputs, l1, l2, idx1_all, oh1_all, idx2_all,
                            oh2_all)
    res = run_bass_kernel_spmd(nc, in_maps, core_ids=list(range(CORES)),
                               trace=trace, tmpdir=tmpdir)
    out = np.concatenate(
        [np.asarray(r["out"][0:NPC], dtype=np.float32) for r in res.results],
        axis=0)
    return out, res


def kernel(**inputs):
    out, _ = _run(inputs, trace=False)
    return out

